# revision 9
# baseline (speedup 1.0000x reference)
"""Trainium2 Bass kernel for nn_Block_35837207118566 (IBP causal attention block).

Math (per batch b):
  qkv   = x @ Wqkv.T ; split q,k,v                       (exact path)
  m = (x_lower+x_upper)/2, d = (x_upper-x_lower)/2
  lo = m@W.T - d@|W|.T ; hi = m@W.T + d@|W|.T            (== reference's pos/neg split, exactly)
  ql,kl,vl / qu,ku,vu from lo/hi
  5 causal softmax prob matrices: (q,k), (ql,kl), (ql,ku), (qu,kl), (qu,ku)
  y      = A_ex @ v
  y_lower= min over 4 interval A of min(A@vl, A@vu);  y_upper analogous with max
  outputs: (y, y_lower, y_upper) each @ Wproj.T

Sharding: 8 cores = (batch b in 0..3) x (head-group g in 0..1, 6 heads each).
Each core computes its head-group's attention and a partial Wproj product;
the host sums the two partials per batch.

On-device layout: scores are computed transposed, S^T[k, q] (k on partitions),
via stationary K^T blocks and moving Q^T, so softmax denominators come from a
ones-column appended to the A@V rhs, and exp(S^T) blocks feed A@V directly as
stationary operands with no transposes. No max-subtraction is needed: score
magnitudes are bounded (|s| < ~10) by construction of the inputs.
"""

import numpy as np
import ml_dtypes
from contextlib import ExitStack

import concourse.bass as bass
import concourse.bacc as bacc
import concourse.tile as tile
from concourse import mybir
from concourse.masks import make_identity, make_upper_triangular

BF16 = mybir.dt.bfloat16
F32 = mybir.dt.float32
bfloat16 = ml_dtypes.bfloat16
MULT = mybir.AluOpType.mult
MIN = mybir.AluOpType.min
MAX = mybir.AluOpType.max
SUB = mybir.AluOpType.subtract
ADD = mybir.AluOpType.add
EXP = mybir.ActivationFunctionType.Exp

B, T, C = 4, 1024, 768
H, D = 12, 64
G = 2                 # head groups (cores per batch)
HPG = H // G          # 6 heads per group
DG = HPG * D          # 384
CT = C // 128         # 6 contraction tiles
TT = T // 128         # 8 sequence tiles
MT = DG // 128        # 3 partition tiles per q/k slab
N_CORES = 8

# U psum layout (2 banks): X0,X1,X2 at 129*X in bank0; X3 at 512; exact at 641.
UOFF = [0, 129, 258, 512]
UEX = 641


def _body(tc):
    nc = tc.nc
    xT = nc.dram_tensor("xT", [C, T], BF16, kind="ExternalInput").ap()
    mT = nc.dram_tensor("mT", [C, T], BF16, kind="ExternalInput").ap()
    dT = nc.dram_tensor("dT", [C, T], BF16, kind="ExternalInput").ap()
    wg = nc.dram_tensor("wg", [C, 3 * DG], BF16, kind="ExternalInput").ap()
    awg = nc.dram_tensor("awg", [C, 3 * DG], BF16, kind="ExternalInput").ap()
    wpT = nc.dram_tensor("wpT", [DG, C], BF16, kind="ExternalInput").ap()
    oy = nc.dram_tensor("oy", [T, C], F32, kind="ExternalOutput").ap()
    ol = nc.dram_tensor("ol", [T, C], F32, kind="ExternalOutput").ap()
    ou = nc.dram_tensor("ou", [T, C], F32, kind="ExternalOutput").ap()

    with ExitStack() as ctx:
        persist = ctx.enter_context(tc.tile_pool(name="persist", bufs=1))

        # persistent slabs
        qk = {}
        for nm in ("qe", "ke", "ql", "qu", "kl", "ku"):
            qk[nm] = persist.tile([128, MT, T], BF16, tag=nm, name=nm)
        vex = persist.tile([128, TT, HPG * 65], BF16, tag="vex")
        vint = persist.tile([128, TT, HPG * 129], BF16, tag="vint")
        ysl = {nm: persist.tile([128, TT, DG], BF16, tag=nm, name=nm)
               for nm in ("ye", "yl", "yu")}
        wps = persist.tile([128, MT, C], BF16, tag="wps")
        nc.sync.dma_start(wps, wpT.rearrange("(a p) c -> p a c", p=128))
        msk = persist.tile([128, 128], BF16, tag="msk")
        make_upper_triangular(nc, msk, val=1.0, diag=True)
        ident = persist.tile([128, 128], BF16, tag="ident")
        make_identity(nc, ident)

        # ---------------- stage 1: projections ----------------
        with tc.tile_pool(name="s1src", bufs=1) as s1src, \
             tc.tile_pool(name="s1sb", bufs=2) as s1sb, \
             tc.tile_pool(name="s1ps", bufs=2, space="PSUM") as s1ps:
            xs = s1src.tile([128, CT, T], BF16, tag="xs")
            ms = s1src.tile([128, CT, T], BF16, tag="ms")
            ds = s1src.tile([128, CT, T], BF16, tag="ds")
            wgs = s1src.tile([128, CT, 3 * DG], BF16, tag="wgs")
            aws = s1src.tile([128, CT, 3 * DG], BF16, tag="aws")
            nc.sync.dma_start(xs, xT.rearrange("(a p) t -> p a t", p=128))
            nc.sync.dma_start(ms, mT.rearrange("(a p) t -> p a t", p=128))
            nc.sync.dma_start(ds, dT.rearrange("(a p) t -> p a t", p=128))
            nc.sync.dma_start(wgs, wg.rearrange("(a p) c -> p a c", p=128))
            nc.sync.dma_start(aws, awg.rearrange("(a p) c -> p a c", p=128))

            def proj_T(dst_ap, src, w, wofs, mt, n0):
                ps = s1ps.tile([128, 512], F32, tag="psA")
                for kt in range(CT):
                    nc.tensor.matmul(
                        ps, lhsT=w[:, kt, wofs + mt * 128:wofs + mt * 128 + 128],
                        rhs=src[:, kt, n0:n0 + 512],
                        start=(kt == 0), stop=(kt == CT - 1))
                return ps

            # transposed q/k slabs (exact + interval)
            for wofs, exact_nm, lo_nm, hi_nm in ((0, "qe", "ql", "qu"),
                                                 (DG, "ke", "kl", "ku")):
                for mt in range(MT):
                    for n0 in range(0, T, 512):
                        pse = proj_T(None, xs, wgs, wofs, mt, n0)
                        nc.vector.tensor_copy(qk[exact_nm][:, mt, n0:n0 + 512], pse)
                        psm = proj_T(None, ms, wgs, wofs, mt, n0)
                        psd = s1ps.tile([128, 512], F32, tag="psB")
                        for kt in range(CT):
                            nc.tensor.matmul(
                                psd, lhsT=aws[:, kt, wofs + mt * 128:wofs + mt * 128 + 128],
                                rhs=ds[:, kt, n0:n0 + 512],
                                start=(kt == 0), stop=(kt == CT - 1))
                        sbd = s1sb.tile([128, 512], F32, tag="sbd")
                        nc.vector.tensor_copy(sbd, psd)
                        nc.vector.tensor_tensor(out=qk[lo_nm][:, mt, n0:n0 + 512],
                                                in0=psm, in1=sbd, op=SUB)
                        nc.vector.tensor_tensor(out=qk[hi_nm][:, mt, n0:n0 + 512],
                                                in0=psm, in1=sbd, op=ADD)

            # natural v slabs
            vex_v = vex.rearrange("p t (h c) -> p t h c", c=65)
            vint_v = vint.rearrange("p t (h c) -> p t h c", c=129)
            nc.vector.memset(vex_v[:, :, :, 64:65], 1.0)
            nc.vector.memset(vint_v[:, :, :, 128:129], 1.0)
            for tt in range(TT):
                psv = s1ps.tile([128, DG], F32, tag="psV", bufs=1)
                psmv = s1ps.tile([128, DG], F32, tag="psMV", bufs=1)
                psdv = s1ps.tile([128, DG], F32, tag="psDV", bufs=1)
                for kt in range(CT):
                    st, sp = (kt == 0), (kt == CT - 1)
                    lx = xs[:, kt, tt * 128:(tt + 1) * 128]
                    lm = ms[:, kt, tt * 128:(tt + 1) * 128]
                    ld = ds[:, kt, tt * 128:(tt + 1) * 128]
                    nc.tensor.matmul(psv, lhsT=lx, rhs=wgs[:, kt, 2 * DG:3 * DG], start=st, stop=sp)
                    nc.tensor.matmul(psmv, lhsT=lm, rhs=wgs[:, kt, 2 * DG:3 * DG], start=st, stop=sp)
                    nc.tensor.matmul(psdv, lhsT=ld, rhs=aws[:, kt, 2 * DG:3 * DG], start=st, stop=sp)
                psv_v = psv.rearrange("p (h c) -> p h c", c=64)
                psmv_v = psmv.rearrange("p (h c) -> p h c", c=64)
                psdv_v = psdv.rearrange("p (h c) -> p h c", c=64)
                nc.vector.tensor_copy(vex_v[:, tt, :, 0:64], psv_v)
                sbdv = s1sb.tile([128, DG], F32, tag="sbdv")
                nc.vector.tensor_copy(sbdv, psdv)
                sbdv_v = sbdv.rearrange("p (h c) -> p h c", c=64)
                nc.vector.tensor_tensor(out=vint_v[:, tt, :, 0:64], in0=psmv_v, in1=sbdv_v, op=SUB)
                nc.vector.tensor_tensor(out=vint_v[:, tt, :, 64:128], in0=psmv_v, in1=sbdv_v, op=ADD)

        # ---------------- stage 2: attention ----------------
        with tc.tile_pool(name="epool", bufs=2) as epool, \
             tc.tile_pool(name="est", bufs=2) as est, \
             tc.tile_pool(name="sps", bufs=2, space="PSUM") as sps, \
             tc.tile_pool(name="sxps", bufs=2, space="PSUM") as sxps, \
             tc.tile_pool(name="ups", bufs=1, space="PSUM") as ups:
            for h in range(HPG):
                po = 64 * (h % 2)
                pt = h // 2
                for qc in range(2):
                    q0 = qc * 512
                    nkb = 4 * (qc + 1)
                    # E tiles: pairs (ll,lu), (ul,uu) and exact
                    e01 = epool.tile([128, nkb, 2, 512], BF16, tag="e01")
                    e23 = epool.tile([128, nkb, 2, 512], BF16, tag="e23")
                    e4 = epool.tile([128, nkb, 512], BF16, tag="e4")
                    for kb in range(nkb):
                        qstart = max(q0, kb * 128)
                        qo = qstart - q0
                        diag = kb * 128 >= q0
                        for epair, qnm in ((e01, "ql"), (e23, "qu")):
                            sp = sps.tile([128, 2, 512], F32, tag="S")
                            for j, knm in enumerate(("kl", "ku")):
                                nc.tensor.matmul(
                                    sp[:, j, qo:512],
                                    lhsT=qk[knm][po:po + 64, pt, kb * 128:(kb + 1) * 128],
                                    rhs=qk[qnm][po:po + 64, pt, qstart:q0 + 512],
                                    start=True, stop=True)
                            nc.scalar.activation(epair[:, kb, :, qo:512], sp[:, :, qo:512], EXP)
                            if diag:
                                mb = bass.AP(tensor=msk.tensor, offset=msk.offset,
                                             ap=msk.ap[:1] + [[0, 2]] + msk.ap[1:])
                                nc.vector.tensor_tensor(out=epair[:, kb, :, qo:qo + 128],
                                                        in0=epair[:, kb, :, qo:qo + 128],
                                                        in1=mb, op=MULT)
                        sx = sxps.tile([128, 512], F32, tag="SX")
                        nc.tensor.matmul(
                            sx[:, qo:512],
                            lhsT=qk["ke"][po:po + 64, pt, kb * 128:(kb + 1) * 128],
                            rhs=qk["qe"][po:po + 64, pt, qstart:q0 + 512],
                            start=True, stop=True)
                        nc.scalar.activation(e4[:, kb, qo:512], sx[:, qo:512], EXP)
                        if diag:
                            nc.vector.tensor_tensor(out=e4[:, kb, qo:qo + 128],
                                                    in0=e4[:, kb, qo:qo + 128],
                                                    in1=msk, op=MULT)

                        # A@V + epilogue once this diagonal block's E is done
                        if not diag:
                            continue
                        qb = kb
                        qbl = qb - 4 * qc
                        u = ups.tile([128, 1024], F32, tag="U")
                        for X in range(4):
                            ep, j = (e01, X) if X < 2 else (e23, X - 2)
                            for kp in range(qb + 1):
                                nc.tensor.matmul(
                                    u[:, UOFF[X]:UOFF[X] + 129],
                                    lhsT=ep[:, kp, j, qbl * 128:qbl * 128 + 128],
                                    rhs=vint_v[:, kp, h, :],
                                    start=(kp == 0), stop=(kp == qb))
                        for kp in range(qb + 1):
                            nc.tensor.matmul(
                                u[:, UEX:UEX + 65],
                                lhsT=e4[:, kp, qbl * 128:qbl * 128 + 128],
                                rhs=vex_v[:, kp, h, :],
                                start=(kp == 0), stop=(kp == qb))

                        # epilogue
                        ru = est.tile([128, 5], F32, tag="ru")
                        u012 = u[:, 0:387].rearrange("p (x c) -> p x c", c=129)
                        nc.vector.reciprocal(ru[:, 0:3], u012[:, :, 128])
                        nc.vector.reciprocal(ru[:, 3:4], u[:, 512 + 128:512 + 129])
                        nc.vector.reciprocal(ru[:, 4:5], u[:, UEX + 64:UEX + 65])
                        vus = est.tile([128, 256], F32, tag="vus")
                        vus_v = vus.rearrange("p (x c) -> p x c", c=64)
                        nc.vector.tensor_copy(vus_v[:, 0:3, :], u012[:, :, 64:128])
                        nc.vector.tensor_copy(vus_v[:, 3:4, :], u[:, 512 + 64:512 + 128])
                        pm = est.tile([128, 256], BF16, tag="pm")
                        px = est.tile([128, 256], BF16, tag="px")
                        pm_v = pm.rearrange("p (x c) -> p x c", c=64)
                        px_v = px.rearrange("p (x c) -> p x c", c=64)
                        nc.vector.tensor_tensor(out=pm_v[:, 0:3, :], in0=u012[:, :, 0:64],
                                                in1=vus_v[:, 0:3, :], op=MIN)
                        nc.vector.tensor_tensor(out=pm_v[:, 3:4, :], in0=u[:, 512:512 + 64],
                                                in1=vus_v[:, 3:4, :], op=MIN)
                        nc.vector.tensor_tensor(out=px_v[:, 0:3, :], in0=u012[:, :, 0:64],
                                                in1=vus_v[:, 0:3, :], op=MAX)
                        nc.vector.tensor_tensor(out=px_v[:, 3:4, :], in0=u[:, 512:512 + 64],
                                                in1=vus_v[:, 3:4, :], op=MAX)
                        sm = est.tile([128, 256], BF16, tag="sm")
                        sx_t = est.tile([128, 256], BF16, tag="sxt")
                        a4 = ru[:, 0:4]
                        rb4 = bass.AP(tensor=a4.tensor, offset=a4.offset,
                                      ap=a4.ap + [[0, 64]])
                        nc.vector.tensor_tensor(out=sm.rearrange("p (x c) -> p x c", c=64),
                                                in0=pm_v, in1=rb4, op=MULT)
                        nc.vector.tensor_tensor(out=sx_t.rearrange("p (x c) -> p x c", c=64),
                                                in0=px_v, in1=rb4, op=MULT)
                        tl = est.tile([128, 128], BF16, tag="tl")
                        tu = est.tile([128, 128], BF16, tag="tu")
                        nc.vector.tensor_tensor(out=tl, in0=sm[:, 0:128], in1=sm[:, 128:256], op=MIN)
                        nc.vector.tensor_tensor(out=tu, in0=sx_t[:, 0:128], in1=sx_t[:, 128:256], op=MAX)
                        nc.vector.tensor_tensor(out=ysl["yl"][:, qb, h * 64:(h + 1) * 64],
                                                in0=tl[:, 0:64], in1=tl[:, 64:128], op=MIN)
                        nc.vector.tensor_tensor(out=ysl["yu"][:, qb, h * 64:(h + 1) * 64],
                                                in0=tu[:, 0:64], in1=tu[:, 64:128], op=MAX)
                        a1 = ru[:, 4:5]
                        rbe = bass.AP(tensor=a1.tensor, offset=a1.offset,
                                      ap=a1.ap[:1] + [[0, 64]])
                        nc.vector.tensor_tensor(out=ysl["ye"][:, qb, h * 64:(h + 1) * 64],
                                                in0=u[:, UEX:UEX + 64], in1=rbe, op=MULT)

        # ---------------- stage 3: output projection ----------------
        with tc.tile_pool(name="s3ps", bufs=2, space="PSUM") as s3ps, \
             tc.tile_pool(name="s3tp", bufs=4, space="PSUM") as s3tp, \
             tc.tile_pool(name="s3sb", bufs=2) as s3sb, \
             tc.tile_pool(name="yTp", bufs=2) as yTp:
            for nm, odram in (("ye", oy), ("yl", ol), ("yu", ou)):
                yT = yTp.tile([128, MT, T], BF16, tag="yT")
                for tt in range(TT):
                    for dt in range(MT):
                        pst = s3tp.tile([128, 128], BF16, tag="pst")
                        nc.tensor.transpose(pst, ysl[nm][:, tt, dt * 128:(dt + 1) * 128], ident)
                        nc.vector.tensor_copy(yT[:, dt, tt * 128:(tt + 1) * 128], pst)
                for tt in range(TT):
                    ost = s3sb.tile([128, C], F32, tag="ost")
                    for n0, nn in ((0, 512), (512, 256)):
                        ps = s3ps.tile([128, 512], F32, tag="ps3")
                        for dt in range(MT):
                            nc.tensor.matmul(ps[:, 0:nn],
                                             lhsT=yT[:, dt, tt * 128:(tt + 1) * 128],
                                             rhs=wps[:, dt, n0:n0 + nn],
                                             start=(dt == 0), stop=(dt == MT - 1))
                        nc.vector.tensor_copy(ost[:, n0:n0 + nn], ps[:, 0:nn])
                    nc.sync.dma_start(odram[tt * 128:(tt + 1) * 128, :], ost)


_NC_CACHE = None


def _build_nc():
    global _NC_CACHE
    if _NC_CACHE is None:
        nc = bacc.Bacc("TRN2", target_bir_lowering=False, debug=False)
        with tile.TileContext(nc) as tc:
            _body(tc)
        nc.compile()
        _NC_CACHE = nc
    return _NC_CACHE


def _prep_inputs(x, x_lower, x_upper, Wqkv, Wproj):
    m = 0.5 * (x_lower.astype(np.float64) + x_upper.astype(np.float64))
    d = 0.5 * (x_upper.astype(np.float64) - x_lower.astype(np.float64))
    m = m.astype(np.float32)
    d = d.astype(np.float32)
    WqkvT = np.ascontiguousarray(Wqkv.T)          # [768, 2304]
    WprojT = np.ascontiguousarray(Wproj.T)        # [768, 768]
    scale = 1.0 / np.sqrt(np.float32(D))
    in_maps = []
    for c in range(N_CORES):
        b, g = c // G, c % G
        sl = slice(g * DG, (g + 1) * DG)
        wg_g = np.concatenate([WqkvT[:, sl],
                               WqkvT[:, C + g * DG:C + (g + 1) * DG] * scale,
                               WqkvT[:, 2 * C + g * DG:2 * C + (g + 1) * DG]], axis=1)
        in_maps.append({
            "xT": np.ascontiguousarray(x[b].T).astype(bfloat16),
            "mT": np.ascontiguousarray(m[b].T).astype(bfloat16),
            "dT": np.ascontiguousarray(d[b].T).astype(bfloat16),
            "wg": wg_g.astype(bfloat16),
            "awg": np.abs(wg_g).astype(bfloat16),
            "wpT": np.ascontiguousarray(WprojT[sl, :]).astype(bfloat16),
        })
    return in_maps


_RUNNER = None


def _get_runner():
    """Build (once) a cached sharded jit callable over the 8 cores.

    Mirrors concourse.bass2jax.run_bass_via_pjrt, but caches the jitted
    function so repeat kernel() calls skip retracing/recompiling.
    """
    global _RUNNER
    if _RUNNER is not None:
        return _RUNNER
    import jax
    from jax.experimental.shard_map import shard_map
    from jax.sharding import Mesh, PartitionSpec
    from concourse import bass2jax as b2j
    from concourse import mybir as _mb

    nc = _build_nc()
    b2j.install_neuronx_cc_hook()
    partition_name = nc.partition_id_tensor.name if nc.partition_id_tensor else None
    in_names, out_names, out_avals, zero_outs = [], [], [], []
    for alloc in nc.m.functions[0].allocations:
        if not isinstance(_mb.MemoryLocationSet, type) or not isinstance(alloc, _mb.MemoryLocationSet):
            continue
        name = alloc.memorylocations[0].name
        if alloc.kind == "ExternalInput":
            if name != partition_name:
                in_names.append(name)
        elif alloc.kind == "ExternalOutput":
            out_names.append(name)
            shape = tuple(alloc.tensor_shape)
            dtype = _mb.dt.np(alloc.dtype)
            out_avals.append(jax.core.ShapedArray(shape, dtype))
            zero_outs.append(np.zeros(shape, dtype))
    n_params = len(in_names)
    n_outs = len(out_avals)
    all_names = in_names + out_names
    if partition_name is not None:
        all_names = all_names + [partition_name]
    donate = tuple(range(n_params, n_params + n_outs))

    def _bodyfn(*args):
        operands = list(args)
        if partition_name is not None:
            operands.append(b2j.partition_id_tensor())
        outs = b2j._bass_exec_p.bind(
            *operands,
            out_avals=tuple(out_avals),
            in_names=tuple(all_names),
            out_names=tuple(out_names),
            lowering_input_output_aliases=(),
            sim_require_finite=True,
            sim_require_nnan=True,
            nc=nc,
        )
        return tuple(outs)

    devices = jax.devices()[:N_CORES]
    mesh = Mesh(np.asarray(devices), ("core",))
    in_specs = (PartitionSpec("core"),) * (n_params + n_outs)
    out_specs = (PartitionSpec("core"),) * n_outs
    sharded = jax.jit(
        shard_map(_bodyfn, mesh=mesh, in_specs=in_specs, out_specs=out_specs,
                  check_rep=False),
        donate_argnums=donate, keep_unused=True)
    _RUNNER = (sharded, in_names, out_names, out_avals, zero_outs)
    return _RUNNER


def _run(in_maps):
    sharded, in_names, out_names, out_avals, zero_outs = _get_runner()
    concat_in = [np.concatenate([in_maps[c][n] for c in range(N_CORES)], axis=0)
                 for n in in_names]
    concat_zeros = [np.zeros((N_CORES * z.shape[0], *z.shape[1:]), z.dtype)
                    for z in zero_outs]
    out_arrs = sharded(*concat_in, *concat_zeros)
    return [{n: np.asarray(out_arrs[i]).reshape(N_CORES, *out_avals[i].shape)[c]
             for i, n in enumerate(out_names)}
            for c in range(N_CORES)]


def kernel(x, x_lower, x_upper, Wqkv, Wproj):
    in_maps = _prep_inputs(x, x_lower, x_upper, Wqkv, Wproj)
    res = _run(in_maps)
    y = np.zeros((B, T, C), np.float32)
    yl = np.zeros((B, T, C), np.float32)
    yu = np.zeros((B, T, C), np.float32)
    for c in range(N_CORES):
        b = c // G
        y[b] += res[c]["oy"]
        yl[b] += res[c]["ol"]
        yu[b] += res[c]["ou"]
    return (y, yl, yu)
